# revision 3
# baseline (speedup 1.0000x reference)
"""Single-head causal attention on 8 Trainium2 NeuronCores.

Problem: B=8, T=2048, C=1024, H=128 (fp32).
    q = x@Wq; k = x@Wk; v = x@Wv
    out = softmax(causal(q k^T / sqrt(H))) @ v

Sharding: data-parallel over batch — core b computes batch element b.

Per-core kernel (matmuls in fp32r, which streams at 1 cyc/row for
free-dim >= 256 vs 4 cyc/row for plain fp32):
  - x is fed pre-transposed from the host as xT [C, T] so the
    contraction dim C lands on SBUF partitions directly.
  - qT, kT, vT [H=128, T] = W^T @ xT   (H on partitions)
  - V [s, H] via PE transpose of vT (needed as matmul lhsT for PV)
  - per 512-wide t-chunk j:
      for s-block i (128 wide, i <= 4j+3):
        S^T [s,t] = kT_i^T @ qT_j        (single matmul, K=H=128)
        diagonal blocks: += additive causal mask (DVE)
        P = exp(scale * S^T)             (ScalarE, PSUM -> SBUF)
        outT_j  += V_i^T @ P             (PSUM accumulate)
        rowsum_j += ones^T @ P           (PSUM accumulate, M=1)
  - outputs: unnormalized outT [128, T] and rowsum [1, T];
    the host divides and transposes (B*T*H fp32 divides, trivial).
"""

import numpy as np

import concourse.bass as bass
import concourse.tile as tile
from concourse import bacc, mybir
from concourse.bass_utils import run_bass_kernel_spmd
from concourse.masks import make_identity

B, T, C, H = 8, 2048, 1024, 128
N_CORES = 8
TCH = 512                # t-chunk width
N_TCH = T // TCH         # 4
SB = 128                 # s-block width
N_SB = T // SB           # 16
KCH = C // 128           # 8 contraction chunks
SCALE = float(H) ** -0.5
MASK_VAL = -1e30

F32 = mybir.dt.float32
F32R = mybir.dt.float32r


def build_graph():
    nc = bacc.Bacc("TRN2", target_bir_lowering=False, debug=False,
                   num_devices=N_CORES)

    xT_d = nc.dram_tensor("xT", [C, T], F32R, kind="ExternalInput").ap()
    wq_d = nc.dram_tensor("Wq", [C, H], F32R, kind="ExternalInput").ap()
    wk_d = nc.dram_tensor("Wk", [C, H], F32R, kind="ExternalInput").ap()
    wv_d = nc.dram_tensor("Wv", [C, H], F32R, kind="ExternalInput").ap()
    outT_d = nc.dram_tensor("outT", [H, T], F32, kind="ExternalOutput").ap()
    rowsum_d = nc.dram_tensor("rowsum", [1, T], F32, kind="ExternalOutput").ap()

    with tile.TileContext(nc) as tc:
        with (
            tc.tile_pool(name="const", bufs=1) as cpool,
            tc.tile_pool(name="sb", bufs=1) as sbpool,
            tc.tile_pool(name="pp", bufs=2, space="PSUM") as pp_pool,
            tc.tile_pool(name="ps", bufs=3, space="PSUM") as ps_pool,
            tc.tile_pool(name="pacc", bufs=2, space="PSUM") as pacc_pool,
            tc.tile_pool(name="prow", bufs=1, space="PSUM") as prow_pool,
            tc.tile_pool(name="pt", bufs=4) as p_pool,
        ):
            # ---- constants -------------------------------------------------
            ident = cpool.tile([128, 128], F32, tag="ident")
            make_identity(nc, ident[:])

            ones_f = cpool.tile([128, 1], F32, tag="ones_f")
            nc.gpsimd.memset(ones_f[:], 1.0)
            ones = cpool.tile([128, 1], F32R, tag="ones")
            nc.vector.tensor_copy(ones[:], ones_f[:])

            # 4 additive causal masks for the diagonal s-blocks.
            # For s-block i = 4j + r inside t-chunk j:
            #   t_local - 128*r - s_local >= 0  -> keep (0), else MASK_VAL
            masks = cpool.tile([128, 4, TCH], F32, tag="masks")
            nc.gpsimd.memset(masks[:], 0.0)
            for rr in range(4):
                nc.gpsimd.affine_select(
                    out=masks[:, rr, :],
                    in_=masks[:, rr, :],
                    compare_op=mybir.AluOpType.is_ge,
                    fill=MASK_VAL,
                    base=-128 * rr,
                    pattern=[[1, TCH]],
                    channel_multiplier=-1,
                )

            # ---- weights ---------------------------------------------------
            w_sb = []
            for name, wd in (("wq", wq_d), ("wk", wk_d), ("wv", wv_d)):
                w = cpool.tile([128, KCH, H], F32R, tag=name)
                nc.sync.dma_start(w[:], wd.rearrange("(k p) h -> p k h", p=128))
                w_sb.append(w)

            # ---- x^T (C on partitions) -------------------------------------
            xT_sb = sbpool.tile([128, KCH, T], F32R, tag="xT")
            xT_r = xT_d.rearrange("(k p) t -> p k t", p=128)
            for j in range(N_TCH):
                nc.sync.dma_start(
                    xT_sb[:, :, j * TCH:(j + 1) * TCH],
                    xT_r[:, :, j * TCH:(j + 1) * TCH],
                )

            qT = sbpool.tile([128, T], F32R, tag="qT")
            kT = sbpool.tile([128, T], F32R, tag="kT")
            vT = sbpool.tile([128, T], F32, tag="vT")
            V = sbpool.tile([128, N_SB, H], F32R, tag="V")
            outT_sb = sbpool.tile([128, T], F32, tag="outT")
            rowsum_sb = sbpool.tile([1, T], F32, tag="rowsum")

            for j in range(N_TCH):
                tsl = slice(j * TCH, (j + 1) * TCH)

                # ---- projections for this t-chunk --------------------------
                for w, dst in ((w_sb[0], qT), (w_sb[1], kT), (w_sb[2], vT)):
                    ps = pp_pool.tile([128, TCH], F32, tag="pp")
                    for k in range(KCH):
                        nc.tensor.matmul(
                            ps[:],
                            w[:, k, :],
                            xT_sb[:, k, tsl],
                            start=(k == 0),
                            stop=(k == KCH - 1),
                        )
                    nc.vector.tensor_copy(dst[:, tsl], ps[:])

                # ---- V blocks for this t-chunk (transpose vT) --------------
                pt = pp_pool.tile([128, TCH], F32, tag="pp")
                for q in range(4):
                    sb = 4 * j + q
                    nc.tensor.transpose(
                        pt[:, q * 128:(q + 1) * 128],
                        vT[:, sb * 128:(sb + 1) * 128],
                        ident[:],
                    )
                nc.vector.tensor_copy(V[:, 4 * j:4 * (j + 1), :], pt[:])

                # ---- attention for this t-chunk ----------------------------
                n_i = 4 * j + 4
                acc = pacc_pool.tile([128, TCH], F32, tag="acc")
                rs = prow_pool.tile([1, TCH], F32, tag="rs")

                P_tiles = {}

                def issue_s(i, j=j, tsl=tsl, P_tiles=P_tiles):
                    S = ps_pool.tile([128, TCH], F32, tag="S")
                    nc.tensor.matmul(
                        S[:],
                        kT[:, i * SB:(i + 1) * SB],
                        qT[:, tsl],
                        start=True,
                        stop=True,
                    )
                    if i >= 4 * j:
                        nc.vector.tensor_add(S[:], S[:], masks[:, i - 4 * j, :])
                    P = p_pool.tile([128, TCH], F32R, tag="P")
                    nc.scalar.activation(
                        P[:], S[:], mybir.ActivationFunctionType.Exp,
                        scale=SCALE,
                    )
                    P_tiles[i] = P

                LOOK = 2
                for l in range(min(LOOK + 1, n_i)):
                    issue_s(l)
                for i in range(n_i):
                    if i + LOOK + 1 < n_i:
                        issue_s(i + LOOK + 1)
                    P = P_tiles.pop(i)
                    nc.tensor.matmul(
                        acc[:], V[:, i, :], P[:],
                        start=(i == 0), stop=(i == n_i - 1),
                    )
                    nc.tensor.matmul(
                        rs[:], ones[:], P[:],
                        start=(i == 0), stop=(i == n_i - 1),
                    )

                nc.vector.tensor_copy(outT_sb[:, tsl], acc[:])
                nc.vector.tensor_copy(rowsum_sb[:, tsl], rs[:])
                nc.sync.dma_start(outT_d[:, tsl], outT_sb[:, tsl])
                nc.sync.dma_start(rowsum_d[:, tsl], rowsum_sb[:, tsl])

    nc.compile()
    return nc


_CACHE = {}


def _get_graph():
    if "nc" not in _CACHE:
        _CACHE["nc"] = build_graph()
    return _CACHE["nc"]


def kernel(x, Wq, Wk, Wv):
    nc = _get_graph()
    x = np.asarray(x, dtype=np.float32)
    wq = np.ascontiguousarray(np.asarray(Wq, dtype=np.float32))
    wk = np.ascontiguousarray(np.asarray(Wk, dtype=np.float32))
    wv = np.ascontiguousarray(np.asarray(Wv, dtype=np.float32))

    in_maps = []
    for b in range(B):
        in_maps.append({
            "xT": np.ascontiguousarray(x[b].T),
            "Wq": wq, "Wk": wk, "Wv": wv,
        })

    res = run_bass_kernel_spmd(nc, in_maps, list(range(N_CORES)))

    outs = np.empty((B, T, H), dtype=np.float32)
    for b in range(B):
        oT = res.results[b]["outT"]          # [H, T]
        rsum = res.results[b]["rowsum"]      # [1, T]
        outs[b] = (oT / rsum).T
    return outs
